# revision 9
# baseline (speedup 1.0000x reference)
"""Trainium2 Bass kernel for nn_Attention_63995012710903.

Math: the reference's mask makes softmax collapse (see below): per (batch,
head), with device positions j = 2047 - q_orig (host reverses the sequence),
e_j = exp(s_jj/8), w_j = 1/(e_j + j):
    z_j = w_j * (S_j + (e_j - 1) v_j),   S_j = sum_{j' <= j} v_{j'}
Only the score diagonal and prefix sums of v are needed.

Layout strategy (per core = (batch b, head-pair g)):
  - q,k in pos-layout (positions on partitions) -> cheap diag + per-position
    scalar chain at [128, few] shapes.
  - v in f-layout (features on partitions, positions on free dim) -> prefix
    sum via the DVE tensor_tensor_scan (no suffix matmuls, no R chain), and
    z^T feeds the W_O matmul directly (no transposes).
  - per-position scalars cross layouts via a tiny PE transpose [128,16] ->
    [16,128] + collapse DMAs -> row vectors [2, 512], then PE indicator
    matmuls broadcast rows into [128, 512] PSUM tiles for the DVE muls.
Each core computes out^T partial [512 dout, 2048 pos] = WoT_blk^T @ z^T; the
host sums the 4 partials per batch, un-reverses and transposes.
"""

import os
import sys

import numpy as np

for _p in ("/opt/trn_rl_repo", "/root/.axon_site/_ro/trn_rl_repo"):
    if os.path.isdir(_p) and _p not in sys.path:
        sys.path.insert(0, _p)

import ml_dtypes  # noqa: E402

import concourse.bass as bass  # noqa: E402
import concourse.tile as tile  # noqa: E402
from concourse import bacc, mybir  # noqa: E402
from concourse.bass_utils import run_bass_kernel_spmd  # noqa: E402


def _install_ntff_hook_shim():
    """antenv.axon_hooks is missing in this image, so the boot-time NTFF
    profile hook registration silently degraded.  Recreate the module and
    register the ctypes hook so trace=True yields exec_time_ns."""
    try:
        import antenv.axon_hooks  # noqa: F401
        return
    except ImportError:
        pass
    try:
        import types

        import antenv
        mod = types.ModuleType("antenv.axon_hooks")
        holder = {}
        mod.set_axon_ntff_profile_hook = lambda h: holder.__setitem__("h", h)
        mod.get_axon_ntff_profile_hook = lambda: holder.get("h")
        sys.modules["antenv.axon_hooks"] = mod
        antenv.axon_hooks = mod
        from trn_agent_boot.trn_boot import _ntff_profile_via_ctypes
        hook = _ntff_profile_via_ctypes("/opt/axon/libaxon_pjrt.so")
        if hook is not None:
            mod.set_axon_ntff_profile_hook(hook)
    except Exception:
        pass


_install_ntff_hook_shim()

BF16 = mybir.dt.bfloat16
F32 = mybir.dt.float32
NPBF16 = ml_dtypes.bfloat16

P = 128
NQ = 4            # quarters
QW = 512          # positions per quarter
SEQ = 2048
DMODEL = 512
NCORES = 8

ADD = mybir.AluOpType.add
MULT = mybir.AluOpType.mult
BYPASS = mybir.AluOpType.bypass
EXP = mybir.ActivationFunctionType.Exp


def _build_nc():
    nc = bacc.Bacc("TRN2", target_bir_lowering=False, debug=False,
                   num_devices=NCORES)

    # xq[q, pd, dj, c] = xT_rev[128*dj + pd, 512*q + c]; 4KB/partition runs
    xq = nc.dram_tensor("xq", [NQ, P, 4, QW], BF16, kind="ExternalInput").ap()
    # wqk[pd, dj, :]: cols = [q_h0|q_h1|k_h0|k_h1] each 64
    wqk = nc.dram_tensor("wqk", [P, 4, 256], BF16, kind="ExternalInput").ap()
    wv = nc.dram_tensor("wv", [P, 4, P], BF16, kind="ExternalInput").ap()
    # woT[f, dout] = W_O[dout, 128 g + f]
    woT = nc.dram_tensor("woT", [P, DMODEL], BF16, kind="ExternalInput").ap()
    ident = nc.dram_tensor("ident", [P, P], BF16, kind="ExternalInput").ap()
    ind = nc.dram_tensor("ind", [2, P], BF16, kind="ExternalInput").ap()
    cnt = nc.dram_tensor("cnt", [P, 16, 2], F32, kind="ExternalInput").ap()
    # out[c, dout, q, col] = out^T[128 c + dout, 512 q + col]
    out = nc.dram_tensor("out", [4, P, NQ, QW], BF16,
                         kind="ExternalOutput").ap()

    with tile.TileContext(nc) as tc:
        _body(tc, out, xq, wqk, wv, woT, ident, ind, cnt)
    nc.compile()
    return nc


def _body(tc, out, xq, wqk, wv, woT, ident, ind, cnt):
    nc = tc.nc

    with (
        tc.tile_pool(name="const", bufs=1) as const,
        tc.tile_pool(name="xpool", bufs=1) as xpool,
        tc.tile_pool(name="qksb", bufs=2) as qksb,
        tc.tile_pool(name="vsb", bufs=2) as vsb,
        tc.tile_pool(name="ssb", bufs=2) as ssb,
        tc.tile_pool(name="scal", bufs=2) as scal,
        tc.tile_pool(name="rowsb", bufs=2) as rowsb,
        tc.tile_pool(name="zsb", bufs=2) as zsb,
        tc.tile_pool(name="osb", bufs=3) as osb,
        tc.tile_pool(name="qkps", bufs=2, space="PSUM") as qkps,
        tc.tile_pool(name="vps", bufs=2, space="PSUM") as vps,
        tc.tile_pool(name="scr", bufs=2, space="PSUM") as scrps,
        tc.tile_pool(name="pout", bufs=2, space="PSUM") as pout,
    ):
        # ---- input DMAs first; no engine ops until data needed ----
        wqksb = const.tile([P, 4, 256], BF16)
        nc.sync.dma_start(wqksb[:], wqk[:])
        wvsb = const.tile([P, 4, P], BF16)
        nc.sync.dma_start(wvsb[:], wv[:])
        xsb = xpool.tile([P, NQ, 4, QW], BF16)
        nc.sync.dma_start(xsb[:, 0, :, 0:256], xq[0, :, :, 0:256])
        nc.sync.dma_start(xsb[:, 0, :, 256:QW], xq[0, :, :, 256:QW])
        cntsb = const.tile([P, 16, 2], F32)
        nc.sync.dma_start(cntsb[:], cnt[:])
        identsb = const.tile([P, P], BF16)
        nc.sync.dma_start(identsb[:], ident[:])
        indsb = const.tile([2, P], BF16)
        nc.sync.dma_start(indsb[:], ind[:])
        woTsb = const.tile([P, DMODEL], BF16)
        nc.sync.dma_start(woTsb[:], woT[:])
        for q in (1, 2, 3):
            nc.sync.dma_start(xsb[:, q, :, :], xq[q])

        S_prev = [None]   # scan carry (bf16 AP into prev S tile)
        st = {}           # per-quarter state

        def pe_qk_pair(Q, pair):
            ps = qkps.tile([P, 2, 256], F32, tag="qk", name=f"qk_{Q}_{pair}")
            for i in (0, 1):
                t = 2 * pair + i
                for dj in range(4):
                    nc.tensor.matmul(
                        ps[:, i, :],
                        xsb[:, Q, dj, P * t:P * t + P],
                        wqksb[:, dj, :],
                        start=(dj == 0), stop=(dj == 3))
            return ps

        def pe_v(Q):
            vp = vps.tile([P, QW], F32, tag="v", name=f"v_{Q}")
            for dj in range(4):
                nc.tensor.matmul(vp[:], wvsb[:, dj, :], xsb[:, Q, dj, :],
                                 start=(dj == 0), stop=(dj == 3))
            return vp

        def evac_front(Q):
            """scalar: k evac + v evac (k cols are 128:256 of each qk tile)"""
            s = st[Q]
            k_sb = qksb.tile([P, 4, P], BF16, tag="ksb", name=f"ksb_{Q}")
            nc.scalar.copy(k_sb[:, 0:2, :], s["qkp"][0][:, :, P:256])
            nc.scalar.copy(k_sb[:, 2:4, :], s["qkp"][1][:, :, P:256])
            v_sb = vsb.tile([P, QW], BF16, tag="vsb", name=f"vsb_{Q}")
            nc.scalar.copy(v_sb[:], s["vp"][:])
            s["k_sb"], s["v_sb"] = k_sb, v_sb

        def diag(Q):
            """DVE: prod from q-psum x k-sbuf; gpsimd: reduce; then chain."""
            s = st[Q]
            prod = scal.tile([P, 4, 2, 64], BF16, tag="prod", name=f"prod_{Q}")
            qv = s["qkp"]
            nc.vector.tensor_tensor(prod[:, 0:2, :, :], qv[0][:, :, 0:P],
                                    s["k_sb"][:, 0:2, :], op=MULT)
            nc.vector.tensor_tensor(prod[:, 2:4, :, :], qv[1][:, :, 0:P],
                                    s["k_sb"][:, 2:4, :], op=MULT)
            s_f = scal.tile([P, 4, 2], F32, tag="s", name=f"s_{Q}")
            nc.vector.tensor_reduce(s_f[:], prod[:],
                                    axis=mybir.AxisListType.X, op=ADD)
            e_f = scal.tile([P, 4, 2], F32, tag="e", name=f"e_{Q}")
            nc.scalar.activation(e_f[:], s_f[:], EXP, scale=0.125)
            z_f = scal.tile([P, 4, 2], F32, tag="zz", name=f"zz_{Q}")
            nc.vector.tensor_tensor(z_f[:], e_f[:],
                                    cntsb[:, 4 * Q:4 * Q + 4, :], op=ADD)
            w_f = scal.tile([P, 4, 2], F32, tag="w", name=f"w_{Q}")
            nc.vector.reciprocal(w_f[:], z_f[:])
            trsrc = scal.tile([P, 4, 4], BF16, tag="trsrc", name=f"trsrc_{Q}")
            for h in (0, 1):
                nc.vector.tensor_scalar_add(trsrc[:, h, :], e_f[:, :, h], -1.0)
                nc.vector.tensor_copy(trsrc[:, 2 + h, :], w_f[:, :, h])
            s["trsrc"] = trsrc

        def pe_trp(Q):
            s = st[Q]
            scratch = scrps.tile([P, QW], F32, tag="scr", name=f"scr_{Q}")
            trp = scratch[0:16, 0:64].bitcast(BF16)
            nc.tensor.transpose(trp, s["trsrc"][:], identsb[:])
            s["scratch"], s["trp"] = scratch, trp

        def rows(Q):
            """DVE evac of trp + collapse DMAs"""
            s = st[Q]
            tr_sb = rowsb.tile([16, P], BF16, tag="trsb", name=f"trsb_{Q}")
            nc.vector.tensor_copy(tr_sb[:], s["trp"])
            em1rows = rowsb.tile([2, QW], BF16, tag="em1r", name=f"em1r_{Q}")
            wrows = rowsb.tile([2, QW], BF16, tag="wr", name=f"wr_{Q}")
            for h in (0, 1):
                nc.sync.dma_start(em1rows[h:h + 1, :], tr_sb[4 * h:4 * h + 4, :])
                nc.sync.dma_start(wrows[h:h + 1, :],
                                  tr_sb[8 + 4 * h:12 + 4 * h, :])
            s["em1rows"], s["wrows"] = em1rows, wrows

        def scan(Q):
            s = st[Q]
            S_sb = ssb.tile([P, QW], BF16, tag="S", name=f"S_{Q}")
            init = 0.0 if Q == 0 else S_prev[0]
            nc.vector.tensor_tensor_scan(S_sb[:], s["v_sb"][:],
                                         xsb[:, 0, 0, :], init,
                                         op0=ADD, op1=BYPASS)
            S_prev[0] = S_sb[:, QW - 1:QW]
            s["S_sb"] = S_sb

        def pe_em1bc(Q):
            s = st[Q]
            nc.tensor.matmul(s["scratch"][:], indsb[:], s["em1rows"][:],
                             start=True, stop=True)

        def combine_m1(Q):
            s = st[Q]
            m1 = ssb.tile([P, QW], BF16, tag="m1", name=f"m1_{Q}")
            nc.vector.tensor_tensor(m1[:], s["v_sb"][:], s["scratch"][:],
                                    op=MULT)
            G = ssb.tile([P, QW], BF16, tag="G", name=f"G_{Q}")
            nc.gpsimd.tensor_tensor(G[:], s["S_sb"][:], m1[:], op=ADD)
            s["G"] = G

        def pe_wbc(Q):
            s = st[Q]
            nc.tensor.matmul(s["scratch"][:], indsb[:], s["wrows"][:],
                             start=True, stop=True)

        def combine_z(Q):
            s = st[Q]
            z_sb = zsb.tile([P, QW], BF16, tag="z", name=f"z_{Q}")
            nc.vector.tensor_tensor(z_sb[:], s["G"][:], s["scratch"][:],
                                    op=MULT)
            s["z_sb"] = z_sb

        def wo(Q):
            s = st[Q]
            for c in range(4):
                po = pout.tile([P, QW], F32, tag="po", name=f"po_{Q}_{c}")
                nc.tensor.matmul(po[:], woTsb[:, P * c:P * c + P],
                                 s["z_sb"][:], start=True, stop=True)
                o_sb = osb.tile([P, QW], BF16, tag="o", name=f"o_{Q}_{c}")
                if c % 2 == 0:
                    nc.vector.tensor_copy(o_sb[:], po[:])
                else:
                    nc.scalar.copy(o_sb[:], po[:])
                nc.sync.dma_start(out[c, :, Q, :], o_sb[:])

        # ---------------- pipeline ----------------
        # bootstrap quarter 0 front
        st[0] = {}
        st[0]["qkp"] = [pe_qk_pair(0, 0), pe_qk_pair(0, 1)]
        st[0]["vp"] = pe_v(0)
        evac_front(0)
        diag(0)
        for Q in range(NQ):
            nxt = Q + 1
            if nxt < NQ:
                st[nxt] = {}
            pe_trp(Q)
            if nxt < NQ:
                st[nxt]["qkp"] = [pe_qk_pair(nxt, 0)]
            rows(Q)
            scan(Q)
            pe_em1bc(Q)
            combine_m1(Q)
            pe_wbc(Q)
            combine_z(Q)
            if nxt < NQ:
                st[nxt]["qkp"].append(pe_qk_pair(nxt, 1))
                st[nxt]["vp"] = pe_v(nxt)
                evac_front(nxt)
            wo(Q)
            if nxt < NQ:
                diag(nxt)


_NC_CACHE = {}


def _get_nc():
    if "nc" not in _NC_CACHE:
        _NC_CACHE["nc"] = _build_nc()
    return _NC_CACHE["nc"]


def _make_in_maps(x, W_Q, W_K, W_V, W_O):
    ident = np.eye(P, dtype=np.float32).astype(NPBF16)
    ind = np.zeros((2, P), np.float32)
    ind[0, 0:64] = 1.0
    ind[1, 64:128] = 1.0
    ind = ind.astype(NPBF16)
    cnt = (np.arange(16, dtype=np.float32)[None, :] * P
           + np.arange(P, dtype=np.float32)[:, None])   # [128, 16] = device j
    cnt = np.ascontiguousarray(
        np.repeat(cnt[:, :, None], 2, axis=2), np.float32)  # [128, 16, 2]

    in_maps = []
    for core in range(NCORES):
        b, g = core // 4, core % 4
        xrev = np.asarray(x[b])[::-1, :]            # reverse positions
        xT = np.ascontiguousarray(xrev.T).astype(NPBF16)   # [512, 2048]
        xq = np.ascontiguousarray(
            xT.reshape(4, P, 4, QW).transpose(2, 1, 0, 3))  # [q, pd, dj, c]
        wq = np.asarray(W_Q[2 * g:2 * g + 2])       # [2, 64, 512]
        wk = np.asarray(W_K[2 * g:2 * g + 2])
        wv = np.asarray(W_V[2 * g:2 * g + 2])
        wqk_full = np.concatenate(
            [wq.reshape(P, DMODEL).T, wk.reshape(P, DMODEL).T],
            axis=1).astype(NPBF16)                  # [512, 256]
        wqk_h = np.ascontiguousarray(
            wqk_full.reshape(4, P, 256))            # [dj, pd, 256] -> dram [pd, dj,]
        wqk_h = np.ascontiguousarray(wqk_h.transpose(1, 0, 2))
        wv_full = wv.reshape(P, DMODEL).T.astype(NPBF16)   # [512, 128]
        wv_h = np.ascontiguousarray(
            wv_full.reshape(4, P, P).transpose(1, 0, 2))
        woT_c = np.ascontiguousarray(
            np.asarray(W_O)[:, P * g:P * (g + 1)].T).astype(NPBF16)  # [128,512]
        in_maps.append({
            "xq": xq, "wqk": wqk_h, "wv": wv_h, "woT": woT_c,
            "ident": ident, "ind": ind, "cnt": cnt,
        })
    return in_maps


def _run(x, W_Q, W_K, W_V, W_O, trace=False, **spmd_kwargs):
    nc = _get_nc()
    in_maps = _make_in_maps(x, W_Q, W_K, W_V, W_O)
    res = run_bass_kernel_spmd(nc, in_maps, core_ids=list(range(NCORES)),
                               trace=trace, **spmd_kwargs)
    # device output: [4 c, 128 dout, 4 q, 512 col] = out^T [512, 2048 (rev)]
    full = []
    for b in range(2):
        acc = None
        for g in range(4):
            arr = res.results[4 * b + g]["out"].astype(np.float32)
            oT = arr.reshape(DMODEL, SEQ)           # [dout, j]
            acc = oT if acc is None else acc + oT
        full.append(acc[:, ::-1].T)                 # un-reverse, [2048, 512]
    return np.stack(full), res


def kernel(x, W_Q, W_K, W_V, W_O):
    full, _ = _run(np.asarray(x), np.asarray(W_Q), np.asarray(W_K),
                   np.asarray(W_V), np.asarray(W_O))
    return full


# revision 11
# speedup vs baseline: 1.0645x; 1.0645x over previous
"""Trainium2 Bass kernel for nn_Attention_63995012710903.

Math: the reference's mask makes softmax collapse (see below): per (batch,
head), with device positions j = 2047 - q_orig (host reverses the sequence),
e_j = exp(s_jj/8), w_j = 1/(e_j + j):
    z_j = w_j * (S_j + (e_j - 1) v_j),   S_j = sum_{j' <= j} v_{j'}
Only the score diagonal and prefix sums of v are needed.

Layout strategy (per core = (batch b, head-pair g)):
  - q,k in pos-layout (positions on partitions) -> cheap diag + per-position
    scalar chain at [128, few] shapes.
  - v in f-layout (features on partitions, positions on free dim) -> prefix
    sum via the DVE tensor_tensor_scan (no suffix matmuls, no R chain), and
    z^T feeds the W_O matmul directly (no transposes).
  - per-position scalars cross layouts via a tiny PE transpose [128,16] ->
    [16,128] + collapse DMAs -> row vectors [2, 512], then PE indicator
    matmuls broadcast rows into [128, 512] PSUM tiles for the DVE muls.
Each core computes out^T partial [512 dout, 2048 pos] = WoT_blk^T @ z^T; the
host sums the 4 partials per batch, un-reverses and transposes.
"""

import os
import sys

import numpy as np

for _p in ("/opt/trn_rl_repo", "/root/.axon_site/_ro/trn_rl_repo"):
    if os.path.isdir(_p) and _p not in sys.path:
        sys.path.insert(0, _p)

import ml_dtypes  # noqa: E402

import concourse.bass as bass  # noqa: E402
import concourse.tile as tile  # noqa: E402
from concourse import bacc, mybir  # noqa: E402
from concourse.bass_utils import run_bass_kernel_spmd  # noqa: E402


def _install_ntff_hook_shim():
    """antenv.axon_hooks is missing in this image, so the boot-time NTFF
    profile hook registration silently degraded.  Recreate the module and
    register the ctypes hook so trace=True yields exec_time_ns."""
    try:
        import antenv.axon_hooks  # noqa: F401
        return
    except ImportError:
        pass
    try:
        import types

        import antenv
        mod = types.ModuleType("antenv.axon_hooks")
        holder = {}
        mod.set_axon_ntff_profile_hook = lambda h: holder.__setitem__("h", h)
        mod.get_axon_ntff_profile_hook = lambda: holder.get("h")
        sys.modules["antenv.axon_hooks"] = mod
        antenv.axon_hooks = mod
        from trn_agent_boot.trn_boot import _ntff_profile_via_ctypes
        hook = _ntff_profile_via_ctypes("/opt/axon/libaxon_pjrt.so")
        if hook is not None:
            mod.set_axon_ntff_profile_hook(hook)
    except Exception:
        pass


_install_ntff_hook_shim()

BF16 = mybir.dt.bfloat16
F32 = mybir.dt.float32
NPBF16 = ml_dtypes.bfloat16

P = 128
NQ = 4            # quarters
QW = 512          # positions per quarter
SEQ = 2048
DMODEL = 512
NCORES = 8

ADD = mybir.AluOpType.add
MULT = mybir.AluOpType.mult
BYPASS = mybir.AluOpType.bypass
EXP = mybir.ActivationFunctionType.Exp


def _build_nc():
    nc = bacc.Bacc("TRN2", target_bir_lowering=False, debug=False,
                   num_devices=NCORES)

    # xq[q, pd, dj, c] = xT_rev[128*dj + pd, 512*q + c]; 4KB/partition runs
    xq = nc.dram_tensor("xq", [NQ, P, 4, QW], BF16, kind="ExternalInput").ap()
    # wqk[pd, dj, :]: cols = [q_h0|q_h1|k_h0|k_h1] each 64
    wqk = nc.dram_tensor("wqk", [P, 4, 256], BF16, kind="ExternalInput").ap()
    wv = nc.dram_tensor("wv", [P, 4, P], BF16, kind="ExternalInput").ap()
    # woT[f, dout] = W_O[dout, 128 g + f]
    woT = nc.dram_tensor("woT", [P, DMODEL], BF16, kind="ExternalInput").ap()
    ident = nc.dram_tensor("ident", [P, P], BF16, kind="ExternalInput").ap()
    ind = nc.dram_tensor("ind", [2, P], BF16, kind="ExternalInput").ap()
    cnt = nc.dram_tensor("cnt", [P, 2, 16], F32, kind="ExternalInput").ap()
    # out[p, c, q, col]: out^T[128 c + p, 512 q + col]  (p-major store)
    out = nc.dram_tensor("out", [P, 4, NQ, QW], BF16,
                         kind="ExternalOutput").ap()

    with tile.TileContext(nc) as tc:
        _body(tc, out, xq, wqk, wv, woT, ident, ind, cnt)
    nc.compile()
    return nc


def _body(tc, out, xq, wqk, wv, woT, ident, ind, cnt):
    nc = tc.nc

    with (
        tc.tile_pool(name="const", bufs=1) as const,
        tc.tile_pool(name="xpool", bufs=1) as xpool,
        tc.tile_pool(name="qksb", bufs=2) as qksb,
        tc.tile_pool(name="vsb", bufs=2) as vsb,
        tc.tile_pool(name="ssb", bufs=2) as ssb,
        tc.tile_pool(name="scal", bufs=2) as scal,
        tc.tile_pool(name="rowsb", bufs=2) as rowsb,
        tc.tile_pool(name="zsb", bufs=2) as zsb,
        tc.tile_pool(name="osb", bufs=2) as osb,
        tc.tile_pool(name="qkps", bufs=1, space="PSUM") as qkps,
        tc.tile_pool(name="vps", bufs=2, space="PSUM") as vps,
        tc.tile_pool(name="scr", bufs=2, space="PSUM") as scrps,
        tc.tile_pool(name="pout", bufs=2, space="PSUM") as pout,
    ):
        # ---- input DMAs; x path on sync DGE, consts on scalar DGE ----
        wqksb = const.tile([P, 4, 256], BF16)
        nc.sync.dma_start(wqksb[:], wqk[:])
        xsb = xpool.tile([P, NQ, 4, QW], BF16)
        nc.sync.dma_start(xsb[:, 0, :, 0:P], xq[0, :, :, 0:P])
        nc.sync.dma_start(xsb[:, 0, :, P:QW], xq[0, :, :, P:QW])
        wvsb = const.tile([P, 4, P], BF16)
        nc.scalar.dma_start(wvsb[:], wv[:])
        cntsb = const.tile([P, 2, 16], F32)
        nc.scalar.dma_start(cntsb[:], cnt[:])
        identsb = const.tile([P, P], BF16)
        nc.scalar.dma_start(identsb[:], ident[:])
        indsb = const.tile([2, P], BF16)
        nc.scalar.dma_start(indsb[:], ind[:])
        woTsb = const.tile([P, DMODEL], BF16)
        nc.scalar.dma_start(woTsb[:], woT[:])
        for q in (1, 2, 3):
            nc.sync.dma_start(xsb[:, q, :, :], xq[q])

        S_prev = [None]   # scan carry (bf16 AP into prev S tile)
        st = {}           # per-quarter state

        def pe_qk(Q, tiles):
            s = st[Q]
            if "qkp" not in s:
                s["qkp"] = qkps.tile([P, 4, 256], F32, tag="qk",
                                     name=f"qk_{Q}")
            ps = s["qkp"]
            for t in tiles:
                for dj in range(4):
                    nc.tensor.matmul(
                        ps[:, t, :],
                        xsb[:, Q, dj, P * t:P * t + P],
                        wqksb[:, dj, :],
                        start=(dj == 0), stop=(dj == 3))

        def pe_v(Q):
            vp = vps.tile([P, QW], F32, tag="v", name=f"v_{Q}")
            for dj in range(4):
                nc.tensor.matmul(vp[:], wvsb[:, dj, :], xsb[:, Q, dj, :],
                                 start=(dj == 0), stop=(dj == 3))
            st[Q]["vp"] = vp

        def evac_k(Q):
            s = st[Q]
            k_sb = qksb.tile([P, 4, P], BF16, tag="ksb", name=f"ksb_{Q}")
            nc.scalar.copy(k_sb[:], s["qkp"][:, :, P:256])
            s["k_sb"] = k_sb

        def evac_v(Q):
            s = st[Q]
            v_sb = vsb.tile([P, QW], BF16, tag="vsb", name=f"vsb_{Q}")
            nc.scalar.copy(v_sb[:], s["vp"][:])
            s["v_sb"] = v_sb

        def diag(Q):
            """prod (h-major) -> reduce -> chain -> trsrc [c-major cols]"""
            s = st[Q]
            prod = scal.tile([P, 2, 4, 64], BF16, tag="prod", name=f"prod_{Q}")
            q_ap = s["qkp"][:, :, 0:P]          # [P, 4t, 128(h,f)]
            q_v = q_ap.rearrange("p t (h f) -> p h t f", h=2)
            k_v = s["k_sb"][:].rearrange("p t (h f) -> p h t f", h=2)
            nc.vector.tensor_tensor(prod[:], q_v, k_v, op=MULT)
            s_f = scal.tile([P, 2, 4], F32, tag="s", name=f"s_{Q}")
            nc.vector.tensor_reduce(s_f[:], prod[:],
                                    axis=mybir.AxisListType.X, op=ADD)
            e_f = scal.tile([P, 2, 4], F32, tag="e", name=f"e_{Q}")
            nc.scalar.activation(e_f[:], s_f[:], EXP, scale=0.125)
            z_f = scal.tile([P, 2, 4], F32, tag="zz", name=f"zz_{Q}")
            nc.vector.tensor_tensor(z_f[:], e_f[:],
                                    cntsb[:, :, 4 * Q:4 * Q + 4], op=ADD)
            w_f = scal.tile([P, 2, 4], F32, tag="w", name=f"w_{Q}")
            nc.vector.reciprocal(w_f[:], z_f[:])
            trsrc = scal.tile([P, 4, 4], BF16, tag="trsrc", name=f"trsrc_{Q}")
            nc.vector.tensor_scalar_add(trsrc[:, 0:2, :], e_f[:], -1.0)
            nc.vector.tensor_copy(trsrc[:, 2:4, :], w_f[:])
            s["trsrc"] = trsrc

        def pe_trp(Q):
            s = st[Q]
            scratch = scrps.tile([P, QW], F32, tag="scr", name=f"scr_{Q}")
            trp = scratch[0:16, 0:64].bitcast(BF16)
            nc.tensor.transpose(trp, s["trsrc"][:], identsb[:])
            s["scratch"], s["trp"] = scratch, trp

        def rows(Q):
            s = st[Q]
            tr_sb = rowsb.tile([16, P], BF16, tag="trsb", name=f"trsb_{Q}")
            nc.vector.tensor_copy(tr_sb[:], s["trp"])
            em1rows = rowsb.tile([2, 4, P], BF16, tag="em1r", name=f"em1r_{Q}")
            wrows = rowsb.tile([2, 4, P], BF16, tag="wr", name=f"wr_{Q}")
            nc.sync.dma_start(em1rows[:], tr_sb[0:8, :])
            nc.sync.dma_start(wrows[:], tr_sb[8:16, :])
            s["em1rows"], s["wrows"] = em1rows, wrows

        def scan(Q):
            s = st[Q]
            S_sb = ssb.tile([P, QW], BF16, tag="S", name=f"S_{Q}")
            init = 0.0 if Q == 0 else S_prev[0]
            nc.vector.tensor_tensor_scan(S_sb[:], s["v_sb"][:],
                                         xsb[:, 0, 0, :], init,
                                         op0=ADD, op1=BYPASS)
            S_prev[0] = S_sb[:, QW - 1:QW]
            s["S_sb"] = S_sb

        def pe_em1bc(Q):
            s = st[Q]
            nc.tensor.matmul(s["scratch"][:], indsb[:], s["em1rows"][:],
                             start=True, stop=True)

        def combine_m1(Q):
            s = st[Q]
            m1 = ssb.tile([P, QW], BF16, tag="m1", name=f"m1_{Q}")
            nc.vector.tensor_tensor(m1[:], s["v_sb"][:], s["scratch"][:],
                                    op=MULT)
            G = ssb.tile([P, QW], BF16, tag="G", name=f"G_{Q}")
            nc.gpsimd.tensor_tensor(G[:], s["S_sb"][:], m1[:], op=ADD)
            s["G"] = G

        def pe_wbc(Q):
            s = st[Q]
            nc.tensor.matmul(s["scratch"][:], indsb[:], s["wrows"][:],
                             start=True, stop=True)

        def combine_z(Q):
            s = st[Q]
            z_sb = zsb.tile([P, QW], BF16, tag="z", name=f"z_{Q}")
            nc.vector.tensor_tensor(z_sb[:], s["G"][:], s["scratch"][:],
                                    op=MULT)
            s["z_sb"] = z_sb

        def pe_wo(Q):
            s = st[Q]
            s["po"] = []
            s["o_sb"] = osb.tile([P, 4, QW], BF16, tag="o", name=f"o_{Q}")
            for c in range(4):
                po = pout.tile([P, QW], F32, tag="po", name=f"po_{Q}_{c}")
                nc.tensor.matmul(po[:], woTsb[:, P * c:P * c + P],
                                 s["z_sb"][:], start=True, stop=True)
                s["po"].append(po)

        def evac_o(Q, c, eng):
            s = st[Q]
            if eng == "v":
                nc.vector.tensor_copy(s["o_sb"][:, c, :], s["po"][c][:])
            else:
                nc.scalar.copy(s["o_sb"][:, c, :], s["po"][c][:])

        def store_o(Q):
            nc.sync.dma_start(out[:, :, Q, :], st[Q]["o_sb"][:])

        # ---------------- pipeline ----------------
        st[0] = {}
        pe_qk(0, (0, 1, 2, 3))
        pe_v(0)
        evac_k(0)
        evac_v(0)
        diag(0)
        for Q in range(NQ):
            nxt = Q + 1
            if nxt < NQ:
                st[nxt] = {}
            # PE: trp(Q) | qk(nxt) t01 | em1bc(Q) | qk t23 | wbc(Q) | v | WO(Q)
            pe_trp(Q)
            if nxt < NQ:
                pe_qk(nxt, (0, 1))
            rows(Q)
            scan(Q)
            pe_em1bc(Q)
            combine_m1(Q)
            if nxt < NQ:
                pe_qk(nxt, (2, 3))
            pe_wbc(Q)
            combine_z(Q)
            if nxt < NQ:
                pe_v(nxt)
                evac_k(nxt)
                evac_v(nxt)
            pe_wo(Q)
            evac_o(Q, 0, "v")
            evac_o(Q, 1, "s")
            if nxt < NQ:
                diag(nxt)
            evac_o(Q, 2, "v")
            evac_o(Q, 3, "s")
            store_o(Q)


_NC_CACHE = {}


def _get_nc():
    if "nc" not in _NC_CACHE:
        _NC_CACHE["nc"] = _build_nc()
    return _NC_CACHE["nc"]


def _make_in_maps(x, W_Q, W_K, W_V, W_O):
    ident = np.eye(P, dtype=np.float32).astype(NPBF16)
    ind = np.zeros((2, P), np.float32)
    ind[0, 0:64] = 1.0
    ind[1, 64:128] = 1.0
    ind = ind.astype(NPBF16)
    cnt = (np.arange(16, dtype=np.float32)[None, :] * P
           + np.arange(P, dtype=np.float32)[:, None])   # [128, 16] = device j
    cnt = np.ascontiguousarray(
        np.repeat(cnt[:, None, :], 2, axis=1), np.float32)  # [128, 2, 16]

    in_maps = []
    for core in range(NCORES):
        b, g = core // 4, core % 4
        xrev = np.asarray(x[b])[::-1, :]            # reverse positions
        xT = np.ascontiguousarray(xrev.T).astype(NPBF16)   # [512, 2048]
        xq = np.ascontiguousarray(
            xT.reshape(4, P, 4, QW).transpose(2, 1, 0, 3))  # [q, pd, dj, c]
        wq = np.asarray(W_Q[2 * g:2 * g + 2])       # [2, 64, 512]
        wk = np.asarray(W_K[2 * g:2 * g + 2])
        wv = np.asarray(W_V[2 * g:2 * g + 2])
        wqk_full = np.concatenate(
            [wq.reshape(P, DMODEL).T, wk.reshape(P, DMODEL).T],
            axis=1).astype(NPBF16)                  # [512, 256]
        wqk_h = np.ascontiguousarray(
            wqk_full.reshape(4, P, 256))            # [dj, pd, 256] -> dram [pd, dj,]
        wqk_h = np.ascontiguousarray(wqk_h.transpose(1, 0, 2))
        wv_full = wv.reshape(P, DMODEL).T.astype(NPBF16)   # [512, 128]
        wv_h = np.ascontiguousarray(
            wv_full.reshape(4, P, P).transpose(1, 0, 2))
        woT_c = np.ascontiguousarray(
            np.asarray(W_O)[:, P * g:P * (g + 1)].T).astype(NPBF16)  # [128,512]
        in_maps.append({
            "xq": xq, "wqk": wqk_h, "wv": wv_h, "woT": woT_c,
            "ident": ident, "ind": ind, "cnt": cnt,
        })
    return in_maps


def _run(x, W_Q, W_K, W_V, W_O, trace=False, **spmd_kwargs):
    nc = _get_nc()
    in_maps = _make_in_maps(x, W_Q, W_K, W_V, W_O)
    res = run_bass_kernel_spmd(nc, in_maps, core_ids=list(range(NCORES)),
                               trace=trace, **spmd_kwargs)
    # device output: [4 c, 128 dout, 4 q, 512 col] = out^T [512, 2048 (rev)]
    full = []
    for b in range(2):
        acc = None
        for g in range(4):
            arr = res.results[4 * b + g]["out"].astype(np.float32)
            arr = arr.reshape(P, 4, SEQ // QW, QW)
            oT = arr.transpose(1, 0, 2, 3).reshape(DMODEL, SEQ)  # [dout, j]
            acc = oT if acc is None else acc + oT
        full.append(acc[:, ::-1].T)                 # un-reverse, [2048, 512]
    return np.stack(full), res


def kernel(x, W_Q, W_K, W_V, W_O):
    full, _ = _run(np.asarray(x), np.asarray(W_Q), np.asarray(W_K),
                   np.asarray(W_V), np.asarray(W_O))
    return full
